# revision 25
# baseline (speedup 1.0000x reference)
"""Trainium2 Bass kernel for a dense pre-LN transformer block.

Problem: B=2, T=2048, C=1024, H=16 heads (d=64), FFN 4x, causal attention.

Parallelization over 8 NeuronCores (single SPMD program, one launch):
  - Attention phase: head-tensor-parallel. Core c computes heads {2c, 2c+1}
    for BOTH batches: LN1 (replicated, bf16 stats), Q/K/V projections,
    causal-block attention with unnormalized softmax (denominator via an
    appended ones-column in V).
  - Two 8-core AllToAlls redistribute attn^T from head-split to
    (batch, token)-split; receiver normalizes by the softmax denominators.
  - Post-A2A phase: core c owns (batch c//4, tokens [c%4*512, ...+512)):
    output projection (split into head-even/head-odd halves so the even
    half overlaps the second AllToAll) + residual, LN2, FFN, residual.

Key implementation choices vs a naive port:
  - All tensor-layout transposes run on the DMA XBAR (dma transpose),
    not the PE array: h and v^T round-trip through DRAM scratch.
  - LN affine (g, beta) is folded into the weight matrices host-side.
  - Constants are packed into 3 DMAs; FFN weights stream with deep
    prefetch; mask multiplies run on the (otherwise idle) GpSimd engine.
"""

import numpy as np
import ml_dtypes

B, T, C = 2, 2048, 1024
H, D = 16, 64
FF = 4 * C
EPS = 1e-5
NCORES = 8
TSL = 512  # tokens owned per core in the post-A2A phase
BT = B * T  # 4096

_CACHE = {}


# --------------------------------------------------------------------------
# device program
# --------------------------------------------------------------------------
def _build_program():
    import concourse.bass as bass
    import concourse.mybir as mybir
    import concourse.tile as tile
    from concourse import bacc

    dt = mybir.dt
    f32, bf16 = dt.float32, dt.bfloat16

    nc = bacc.Bacc("TRN2", target_bir_lowering=False, debug=False,
                   num_devices=NCORES)

    # ---- I/O ----
    xbf = nc.dram_tensor("xbf", [BT, C], bf16, kind="ExternalInput")
    x_own = nc.dram_tensor("x_own", [TSL, C], f32, kind="ExternalInput")
    wq2 = nc.dram_tensor("wq2", [C, 128], bf16, kind="ExternalInput")
    wk2 = nc.dram_tensor("wk2", [C, 128], bf16, kind="ExternalInput")
    wv2 = nc.dram_tensor("wv2", [C, 128], bf16, kind="ExternalInput")
    masks = nc.dram_tensor("masks", [128, 4, 512], bf16, kind="ExternalInput")
    wproj = nc.dram_tensor("wproj", [C, C], bf16, kind="ExternalInput")
    w1 = nc.dram_tensor("w1", [C, FF], bf16, kind="ExternalInput")
    w2 = nc.dram_tensor("w2", [FF, C], bf16, kind="ExternalInput")
    b1t = nc.dram_tensor("b1t", [128, FF // 128], f32, kind="ExternalInput")
    # rows blob: [ones(128) | bproj(1024) | b2row(1024)] bf16
    rows = nc.dram_tensor("rows", [1, 128 + C + C], bf16, kind="ExternalInput")
    out = nc.dram_tensor("out", [TSL, C], f32, kind="ExternalOutput")

    with tile.TileContext(nc, num_cores=NCORES) as tc:
        _body(nc, tc, tile, mybir, bass, locals())
    nc.compile()
    return nc


def _body(nc, tc, tile, mybir, bass, io):
    dt = mybir.dt
    f32, bf16 = dt.float32, dt.bfloat16
    AF = mybir.ActivationFunctionType
    OP = mybir.AluOpType

    xbf, x_own = io["xbf"], io["x_own"]
    wq2, wk2, wv2 = io["wq2"], io["wk2"], io["wv2"]
    masks, wproj = io["masks"], io["wproj"]
    w1, w2, b1t = io["w1"], io["w2"], io["b1t"]
    rows, out = io["rows"], io["out"]

    # ---- pools (released LIFO: persA first, dram last) ----
    dram = tc.alloc_tile_pool(name="dram", bufs=1, space="DRAM")
    consts = tc.alloc_tile_pool(name="consts", bufs=1)
    prefD = tc.alloc_tile_pool(name="prefD", bufs=1)   # phase-D persistents
    persA = tc.alloc_tile_pool(name="persA", bufs=1)   # attention lifetime

    # ---- phase-D prefetch ----
    wp_sb = prefD.tile([128, 8, C], bf16, name="wp_sb")
    xo = prefD.tile([128, 4, C], f32, name="xo")
    aT_own = prefD.tile([128, 8, 512], bf16, name="aT_own")
    rb = prefD.tile([128, 8, 512], bf16, name="rb")

    # ---- consts ----
    b1_sb = consts.tile([128, FF // 128], f32, name="b1_sb")
    rows_sb = consts.tile([1, 128 + C + C], bf16, name="rows_sb")
    mask_sb = consts.tile([128, 4, 512], bf16, name="mask_sb")
    wq_sb = consts.tile([128, 8, 128], bf16, name="wq_sb")
    wk_sb = consts.tile([128, 8, 128], bf16, name="wk_sb")
    wv_sb = consts.tile([128, 8, 128], bf16, name="wv_sb")
    eps_sb = consts.tile([128, 1], f32, name="eps_sb")
    ones_b = rows_sb[0:1, 0:128]
    bproj_r = rows_sb[0:1, 128:128 + C]
    b2_r = rows_sb[0:1, 128 + C:128 + 2 * C]

    def load_consts():
        nc.sync.dma_start(out=b1_sb[:], in_=b1t[:])
        nc.sync.dma_start(out=rows_sb[:], in_=rows[:])
        nc.sync.dma_start(out=mask_sb[:], in_=masks[:])
        nc.sync.dma_start(out=wq_sb[:],
                          in_=wq2[:].rearrange("(cc p) d -> p cc d", p=128))
        nc.sync.dma_start(out=wk_sb[:],
                          in_=wk2[:].rearrange("(cc p) d -> p cc d", p=128))
        nc.sync.dma_start(out=wv_sb[:],
                          in_=wv2[:].rearrange("(cc p) d -> p cc d", p=128))
        nc.vector.memset(eps_sb[:], EPS)

    # ---- attention persistents ----
    qT = [persA.tile([128, T], bf16, name=f"qTb{b}") for b in range(2)]
    kT = [persA.tile([128, T], bf16, name=f"kTb{b}") for b in range(2)]
    vaug = [persA.tile([128, 16, 130], bf16, name=f"vaugb{b}") for b in range(2)]
    aT_h = [[persA.tile([64, T], bf16, name=f"aTb{b}h{h}") for h in range(2)]
            for b in range(2)]
    dens = [persA.tile([65, 2, T], bf16, name=f"den_{b}") for b in range(2)]

    # DRAM scratch
    hdram = [[dram.tile([1024, C], bf16, name=f"hdram_{b}_{t}") for t in range(2)]
             for b in range(2)]
    vTdram = [dram.tile([128, T], bf16, name=f"vTdram_{b}") for b in range(2)]
    h2dram = [dram.tile([256, C], bf16, name=f"h2dram_{i}") for i in range(2)]
    denr = [dram.tile([1, 8, 512], bf16, name=f"denr_{h}") for h in range(2)]
    a2a_in = [dram.tile([8, 65, 512], bf16, name=f"a2a_in{h}") for h in range(2)]
    a2a_out = [dram.tile([8, 65, 512], bf16, name=f"a2a_out{h}")
               for h in range(2)]

    # ======================================================================
    # Phase A
    # ======================================================================
    with tc.tile_pool(name="lnA", bufs=1) as lnA, \
         tc.tile_pool(name="psA", bufs=1, space="PSUM") as psA:
        # ones columns of vaug (64 and 129)
        for b in range(2):
            nc.vector.memset(vaug[b][:, :, 64:65], 1.0)
            nc.vector.memset(vaug[b][:, :, 129:130], 1.0)

        # ---- LN1 -> h -> DRAM -> XBAR transpose -> QKV ----
        def qkv_chunk(b, tch):
            th, tl = tch // 2, tch % 2
            with nc.named_scope(f"qkv_b{b}t{tch}"):
                    row0 = b * T + tch * 512
                    xt = lnA.tile([128, 4, C], bf16, tag="xt", bufs=3,
                                  name=f"xt_{b}_{tch}")
                    nc.sync.dma_start(
                        out=xt[:],
                        in_=xbf[row0:row0 + 512, :].rearrange(
                            "(s p) c -> p s c", p=128))
                    if b == 0 and tch == 0:
                        load_consts()
                    ht = lnA.tile([128, 4, C], bf16, tag="ht", bufs=2,
                                  name=f"ht_{b}_{tch}")
                    for sub in range(4):
                        st = lnA.tile([128, 2, 6], f32, tag="st", bufs=2,
                                      name=f"st_{b}_{tch}_{sub}")
                        nc.vector.bn_stats(out=st[:, 0, :],
                                           in_=xt[:, sub, 0:512])
                        nc.vector.bn_stats(out=st[:, 1, :],
                                           in_=xt[:, sub, 512:1024])
                        mv = lnA.tile([128, 2], f32, tag="mv", bufs=2,
                                      name=f"mv_{b}_{tch}_{sub}")
                        nc.vector.bn_aggr(out=mv[:], in_=st[:])
                        rs = lnA.tile([128, 1], f32, tag="rs", bufs=2,
                                      name=f"rs_{b}_{tch}_{sub}")
                        nc.scalar.activation(out=rs[:], in_=mv[:, 1:2],
                                             func=AF.Sqrt, bias=eps_sb[:])
                        nc.vector.reciprocal(out=rs[:], in_=rs[:])
                        nmr = lnA.tile([128, 1], f32, tag="nmr", bufs=2,
                                       name=f"nmr_{b}_{tch}_{sub}")
                        nc.vector.tensor_scalar(
                            out=nmr[:], in0=mv[:, 0:1], scalar1=rs[:],
                            scalar2=-1.0, op0=OP.mult, op1=OP.mult)
                        nc.scalar.activation(
                            out=ht[:, sub, :], in_=xt[:, sub, :],
                            func=AF.Identity, bias=nmr[:], scale=rs[:])
                    nc.sync.dma_start(
                        out=hdram[b][th][tl * 512:tl * 512 + 512, :].rearrange(
                            "(s p) c -> p s c", p=128),
                        in_=ht[:])
                    # XBAR transpose: [512, 1024] -> [128, 8cc, 512] in one go
                    hTb = lnA.tile([128, 8, 512], bf16, tag="hTb", bufs=3,
                                   name=f"hTb_{b}_{tch}")
                    nc.sync.dma_start(
                        out=hTb[:],
                        in_=hdram[b][th][tl * 512:tl * 512 + 512, :],
                        transpose=True)
                    # q^T, k^T, v^T for this 512-token chunk
                    col = tch * 512
                    for w_sb, dstT in ((wq_sb, qT[b]), (wk_sb, kT[b])):
                        pqk = psA.tile([128, 512], f32, tag="pqk", bufs=2,
                                       name=f"pqk_{b}_{tch}_{dstT.name}")
                        for cc in range(8):
                            nc.tensor.matmul(pqk[:], w_sb[:, cc, :],
                                             hTb[:, cc, :],
                                             start=(cc == 0), stop=(cc == 7))
                        nc.vector.tensor_copy(out=dstT[:, col:col + 512],
                                              in_=pqk[:])
                    pvt = psA.tile([128, 512], f32, tag="pqk", bufs=2,
                                   name=f"pvt_{b}_{tch}")
                    for cc in range(8):
                        nc.tensor.matmul(pvt[:], wv_sb[:, cc, :],
                                         hTb[:, cc, :],
                                         start=(cc == 0), stop=(cc == 7))
                    vts = lnA.tile([128, 512], bf16, tag="vts", bufs=2,
                                   name=f"vts_{b}_{tch}")
                    nc.vector.tensor_copy(out=vts[:], in_=pvt[:])
                    nc.sync.dma_start(out=vTdram[b][:, col:col + 512],
                                      in_=vts[:])
                    # v back to token-major via XBAR, into vaug (65-strided)
                    vtmp = lnA.tile([128, 4, 128], bf16, tag="vtmp", bufs=2,
                                    name=f"vtmp_{b}_{tch}")
                    nc.sync.dma_start(out=vtmp[:],
                                      in_=vTdram[b][:, col:col + 512],
                                      transpose=True)
                    nc.vector.tensor_copy(
                        out=vaug[b][:, tch * 4:tch * 4 + 4, :].rearrange(
                            "p s (h x) -> p s h x", h=2)[:, :, :, 0:64],
                        in_=vtmp[:].rearrange("p s (h d) -> p s h d", h=2))
        # ---- attention, h-major; A2A#0 overlaps h=1 attention ----
        def receiver_chain(h):
            hp = 64 * h
            nc.sync.dma_start(
                out=aT_own[hp:hp + 64, :, :],
                in_=a2a_out[h][:, 0:64, :].rearrange("r d t -> d r t"))
            dsb = lnA.tile([8, 512], bf16, tag="dsb", bufs=2, name=f"dsb_{h}")
            nc.sync.dma_start(
                out=dsb[:],
                in_=a2a_out[h][:, 64:65, :].rearrange("r o t -> r (o t)"))
            with nc.allow_low_precision(reason="softmax denom recip in bf16"):
                nc.vector.reciprocal(out=dsb[:], in_=dsb[:])
            nc.sync.dma_start(out=denr[h][0, :, :], in_=dsb[:])
            nc.sync.dma_start(
                out=rb[hp:hp + 64, :, :],
                in_=denr[h][:].broadcast_to([64, 8, 512]))
            nc.vector.tensor_mul(aT_own[hp:hp + 64, :, :],
                                 aT_own[hp:hp + 64, :, :],
                                 rb[hp:hp + 64, :, :])

        def attn_half(h, b, half):
            hp = 64 * h
            den = dens[b]
            with nc.named_scope(f"attn_b{b}h{h}f{half}"):
                qc0 = half * 1024
                pat = [psA.tile([65, 512], f32, tag="pat", bufs=2,
                                name=f"pat_{b}_{h}_{half}_{i}")
                       for i in range(2)]
                nsb = 8 * half + 8
                for sb in range(nsb):
                    act0 = 0 if sb < 8 * half + 4 else 1
                    dtc = sb // 4 - 2 * half
                    ps = psA.tile([128, 1024], f32, tag="ps", bufs=2,
                                  name=f"ps_{b}_{h}_{half}_{sb}")
                    for i in range(act0, 2):
                        nc.tensor.matmul(
                            ps[:, i * 512:(i + 1) * 512],
                            kT[b][hp:hp + 64, sb * 128:sb * 128 + 128],
                            qT[b][hp:hp + 64,
                                  qc0 + i * 512:qc0 + (i + 1) * 512],
                            start=True, stop=True)
                    pt = lnA.tile([128, 1024], bf16, tag="pt", bufs=3,
                                  name=f"pt_{b}_{h}_{half}_{sb}")
                    nc.scalar.activation(out=pt[:, act0 * 512:1024],
                                         in_=ps[:, act0 * 512:1024],
                                         func=AF.Exp, scale=0.125)
                    if dtc >= act0:
                        nc.vector.tensor_mul(
                            pt[:, dtc * 512:(dtc + 1) * 512],
                            pt[:, dtc * 512:(dtc + 1) * 512],
                            mask_sb[:, sb % 4, :])
                    vs = h * 65
                    for i in range(act0, 2):
                        last = 8 * half + 3 if i == 0 else nsb - 1
                        nc.tensor.matmul(
                            pat[i][:], vaug[b][:, sb, vs:vs + 65],
                            pt[:, i * 512:(i + 1) * 512],
                            start=(sb == 0), stop=(sb == last))
                for i in range(2):
                    qcol = (2 * half + i) * 512
                    nc.vector.tensor_copy(
                        out=aT_h[b][h][:, qcol:qcol + 512],
                        in_=pat[i][0:64, :])
                    nc.vector.tensor_copy(
                        out=den[64:65, h, qcol:qcol + 512],
                        in_=pat[i][64:65, :])

        def stage(h, b):
            nc.sync.dma_start(
                out=a2a_in[h][4 * b:4 * b + 4, 0:64, :].rearrange(
                    "q d t -> d q t"),
                in_=aT_h[b][h][:].rearrange("d (q t) -> d q t", q=4))
            nc.sync.dma_start(
                out=a2a_in[h][4 * b:4 * b + 4, 64:65, :].rearrange(
                    "q o t -> o q t"),
                in_=dens[b][64:65, h, :].rearrange("o (q t) -> o q t", q=4))

        # QKV interleaved with h=0 attention halves (fills PE bubbles
        # while LN1 chains run on vector/scalar)
        for b in range(2):
            qkv_chunk(b, 0)
            qkv_chunk(b, 1)
            attn_half(0, b, 0)
            qkv_chunk(b, 2)
            if b == 0:
                # phase-D weight prefetch: transfers during attention
                nc.sync.dma_start(
                    out=wp_sb[:],
                    in_=wproj[:].rearrange("(dc p) e -> p dc e", p=128))
                nc.sync.dma_start(
                    out=xo[:],
                    in_=x_own[:].rearrange("(tq p) e -> p tq e", p=128))
            qkv_chunk(b, 3)
            attn_half(0, b, 1)
            stage(0, b)
        nc.gpsimd.collective_compute(
            "AllToAll", mybir.AluOpType.bypass,
            replica_groups=[list(range(NCORES))],
            ins=[a2a_in[0][:].opt()], outs=[a2a_out[0][:].opt()])
        attn_half(1, 0, 0)
        attn_half(1, 0, 1)
        stage(1, 0)
        receiver_chain(0)  # A2A#0 landed during h1b0 attention
        attn_half(1, 1, 0)
        attn_half(1, 1, 1)
        stage(1, 1)
        nc.gpsimd.collective_compute(
            "AllToAll", mybir.AluOpType.bypass,
            replica_groups=[list(range(NCORES))],
            ins=[a2a_in[1][:].opt()], outs=[a2a_out[1][:].opt()])
        receiver_chain(1)

    # ---- projection (head-even half overlaps A2A#1) ----
    psP = tc.alloc_tile_pool(name="psP", bufs=1, space="PSUM")
    pp = [[psP.tile([128, 512], f32, tag="pp", bufs=8,
                    name=f"pp_{tq}_{eh}") for eh in range(2)]
          for tq in range(4)]
    for h in range(2):
        hp = 64 * h
        with nc.named_scope(f"proj_h{h}"):
            for tq in range(4):
                for eh in range(2):
                    for dc in range(8):
                        nc.tensor.matmul(
                            pp[tq][eh][:],
                            aT_own[hp:hp + 64, dc, tq * 128:(tq + 1) * 128],
                            wp_sb[hp:hp + 64, dc, eh * 512:eh * 512 + 512],
                            start=(h == 0 and dc == 0), stop=False)
                    if h == 1:
                        nc.tensor.matmul(
                            pp[tq][eh][:], ones_b,
                            bproj_r[0:1, eh * 512:eh * 512 + 512],
                            start=False, stop=True)
    persA.release()

    # ======================================================================
    # Phase D: residual + LN2 + FFN + residual
    # ======================================================================
    persD = tc.alloc_tile_pool(name="persD", bufs=1)
    x2 = persD.tile([128, 4, C], f32, name="x2")
    h2T = persD.tile([128, 8, 512], bf16, name="h2T")
    ff1T = persD.tile([128, 32, 512], bf16, name="ff1T")
    w1r = w1[:].rearrange("(cc p) m -> p cc m", p=128)
    with tc.tile_pool(name="prD", bufs=1) as prD:
        with nc.named_scope("ln2"):
            for tq in range(4):
                for eh in range(2):
                    nc.vector.tensor_add(x2[:, tq, eh * 512:(eh + 1) * 512],
                                         pp[tq][eh][:],
                                         xo[:, tq, eh * 512:(eh + 1) * 512])
                st2 = prD.tile([128, 2, 6], f32, tag="st2", bufs=2,
                               name=f"st2_{tq}")
                nc.vector.bn_stats(out=st2[:, 0, :], in_=x2[:, tq, 0:512])
                nc.vector.bn_stats(out=st2[:, 1, :], in_=x2[:, tq, 512:1024])
                mv2 = prD.tile([128, 2], f32, tag="mv2", bufs=2,
                               name=f"mv2_{tq}")
                nc.vector.bn_aggr(out=mv2[:], in_=st2[:])
                rs2 = prD.tile([128, 1], f32, tag="rs2", bufs=2,
                               name=f"rs2_{tq}")
                nc.scalar.activation(out=rs2[:], in_=mv2[:, 1:2], func=AF.Sqrt,
                                     bias=eps_sb[:])
                nc.vector.reciprocal(out=rs2[:], in_=rs2[:])
                h2 = prD.tile([128, C], bf16, tag="h2", bufs=2, name=f"h2_{tq}")
                nc.vector.tensor_scalar(out=h2[:], in0=x2[:, tq, :],
                                        scalar1=mv2[:, 0:1], scalar2=rs2[:],
                                        op0=OP.subtract, op1=OP.mult)
                nc.sync.dma_start(
                    out=h2dram[tq // 2][(tq % 2) * 128:(tq % 2) * 128 + 128, :],
                    in_=h2[:])
                if tq % 2 == 1:
                    h2Ttmp = prD.tile([128, 8, 256], bf16, tag="h2Ttmp",
                                      bufs=2, name=f"h2Ttmp_{tq}")
                    nc.scalar.dma_start(out=h2Ttmp[:],
                                        in_=h2dram[tq // 2][:],
                                        transpose=True)
                    nc.vector.tensor_copy(
                        out=h2T[:, :, (tq // 2) * 256:(tq // 2) * 256 + 256],
                        in_=h2Ttmp[:])
        psP.release()
        with tc.tile_pool(name="ps1", bufs=1, space="PSUM") as ps1, \
             nc.named_scope("ffn1"):
            for w in range(16):  # m-windows of 256
                w1w = prD.tile([128, 8, 256], bf16, tag="w1w", bufs=3,
                               name=f"w1w_{w}")
                nc.sync.dma_start(out=w1w[:],
                                  in_=w1r[:, :, w * 256:(w + 1) * 256])
                for m2 in range(2):
                    m = w * 2 + m2
                    pf = ps1.tile([128, 512], f32, tag="pf", bufs=3,
                                  name=f"pf_{m}")
                    for cc in range(8):
                        nc.tensor.matmul(
                            pf[:], w1w[:, cc, m2 * 128:(m2 + 1) * 128],
                            h2T[:, cc, :], start=(cc == 0), stop=(cc == 7))
                    nc.scalar.activation(out=ff1T[:, m, :], in_=pf[:],
                                         func=AF.Relu, bias=b1_sb[:, m:m + 1])
        with tc.tile_pool(name="ps2", bufs=1, space="PSUM") as ps2p, \
             nc.named_scope("ffn2"):
            pso = [ps2p.tile([128, C], f32, tag="pso", bufs=4, name=f"pso_{tq}")
                   for tq in range(4)]
            for mc in range(32):
                w2t = prD.tile([128, C], bf16, tag="w2t", bufs=8,
                               name=f"w2t_{mc}")
                nc.sync.dma_start(out=w2t[:],
                                  in_=w2[mc * 128:(mc + 1) * 128, :])
                for tq in range(4):
                    for eh in range(2):
                        nc.tensor.matmul(pso[tq][:, eh * 512:(eh + 1) * 512],
                                         ff1T[:, mc, tq * 128:(tq + 1) * 128],
                                         w2t[:, eh * 512:(eh + 1) * 512],
                                         start=(mc == 0), stop=False)
            for tq in range(4):
                for eh in range(2):
                    nc.tensor.matmul(pso[tq][:, eh * 512:(eh + 1) * 512],
                                     ones_b, b2_r[0:1, eh * 512:(eh + 1) * 512],
                                     start=False, stop=True)
                ot = prD.tile([128, C], f32, tag="ot", bufs=2, name=f"ot_{tq}")
                nc.vector.tensor_add(ot[:], pso[tq][:], x2[:, tq, :])
                nc.sync.dma_start(out=out[tq * 128:(tq + 1) * 128, :],
                                  in_=ot[:])
    persD.release()
    prefD.release()
    consts.release()
    dram.release()


# --------------------------------------------------------------------------
# host driver
# --------------------------------------------------------------------------
def _make_in_maps(inputs):
    bf = ml_dtypes.bfloat16
    x = np.ascontiguousarray(np.asarray(inputs["x"], np.float32))
    wq = np.asarray(inputs["wq"], np.float32)
    wk = np.asarray(inputs["wk"], np.float32)
    wv = np.asarray(inputs["wv"], np.float32)
    w_proj = np.asarray(inputs["w_proj"], np.float32)
    b_proj = np.asarray(inputs["b_proj"], np.float32)
    w1 = np.asarray(inputs["w1"], np.float32)
    b1 = np.asarray(inputs["b1"], np.float32)
    w2 = np.asarray(inputs["w2"], np.float32)
    b2 = np.asarray(inputs["b2"], np.float32)
    g1 = np.asarray(inputs["g1"], np.float32)
    be1 = np.asarray(inputs["be1"], np.float32)
    g2 = np.asarray(inputs["g2"], np.float32)
    be2 = np.asarray(inputs["be2"], np.float32)

    assert np.max(np.abs(be1)) == 0.0, "be1 != 0 unsupported in this build"
    xf = x.reshape(BT, C)
    i_mask = np.zeros((128, 4, 512), np.float32)
    s_idx = np.arange(128)[:, None]
    t_idx = np.arange(512)[None, :]
    for i in range(4):
        i_mask[:, i, :] = (s_idx + 128 * i <= t_idx).astype(np.float32)

    # fold LN affine into weights
    wq_g = g1[None, :, None] * wq  # (H, C, D) scaled along C
    wk_g = g1[None, :, None] * wk
    wv_g = g1[None, :, None] * wv
    w1_g = g2[:, None] * w1
    b1_eff = b1 + be2 @ w1

    rows_blob = np.concatenate(
        [np.ones(128, np.float32), b_proj, b2]).astype(bf)[None, :]

    common = dict(
        xbf=xf.astype(bf),
        masks=i_mask.astype(bf),
        wproj=w_proj.astype(bf),
        w1=w1_g.astype(bf), w2=w2.astype(bf),
        b1t=np.ascontiguousarray(b1_eff.reshape(FF // 128, 128).T),
        rows=np.ascontiguousarray(rows_blob),
    )
    in_maps = []
    for c in range(NCORES):
        b, hg = c // 4, c % 4
        m = dict(common)
        m["x_own"] = np.ascontiguousarray(
            xf[b * T + hg * TSL: b * T + (hg + 1) * TSL])
        m["wq2"] = np.ascontiguousarray(
            np.concatenate([wq_g[2 * c], wq_g[2 * c + 1]], axis=1)).astype(bf)
        m["wk2"] = np.ascontiguousarray(
            np.concatenate([wk_g[2 * c], wk_g[2 * c + 1]], axis=1)).astype(bf)
        m["wv2"] = np.ascontiguousarray(
            np.concatenate([wv_g[2 * c], wv_g[2 * c + 1]], axis=1)).astype(bf)
        in_maps.append(m)
    return in_maps


LAST_RESULTS = None


def kernel(trace=False, **inputs):
    global LAST_RESULTS
    from concourse import bass_utils

    if "nc" not in _CACHE:
        _CACHE["nc"] = _build_program()
    nc = _CACHE["nc"]
    in_maps = _make_in_maps(inputs)
    res = bass_utils.run_bass_kernel_spmd(
        nc, in_maps, core_ids=list(range(NCORES)), trace=trace)
    LAST_RESULTS = res
    out = np.zeros((B, T, C), np.float32)
    for c in range(NCORES):
        b, hg = c // 4, c % 4
        out[b, hg * TSL:(hg + 1) * TSL, :] = res.results[c]["out"]
    return out


# revision 26
# speedup vs baseline: 1.0061x; 1.0061x over previous
"""Trainium2 Bass kernel for a dense pre-LN transformer block.

Problem: B=2, T=2048, C=1024, H=16 heads (d=64), FFN 4x, causal attention.

Parallelization over 8 NeuronCores (single SPMD program, one launch):
  - Attention phase: head-tensor-parallel. Core c computes heads {2c, 2c+1}
    for BOTH batches: LN1 (replicated, bf16 stats), Q/K/V projections,
    causal-block attention with unnormalized softmax (denominator via an
    appended ones-column in V).
  - Two 8-core AllToAlls redistribute attn^T from head-split to
    (batch, token)-split; receiver normalizes by the softmax denominators.
  - Post-A2A phase: core c owns (batch c//4, tokens [c%4*512, ...+512)):
    output projection (split into head-even/head-odd halves so the even
    half overlaps the second AllToAll) + residual, LN2, FFN, residual.

Key implementation choices vs a naive port:
  - All tensor-layout transposes run on the DMA XBAR (dma transpose),
    not the PE array: h and v^T round-trip through DRAM scratch.
  - LN affine (g, beta) is folded into the weight matrices host-side.
  - Constants are packed into 3 DMAs; FFN weights stream with deep
    prefetch; mask multiplies run on the (otherwise idle) GpSimd engine.
"""

import numpy as np
import ml_dtypes

B, T, C = 2, 2048, 1024
H, D = 16, 64
FF = 4 * C
EPS = 1e-5
NCORES = 8
TSL = 512  # tokens owned per core in the post-A2A phase
BT = B * T  # 4096

_CACHE = {}


# --------------------------------------------------------------------------
# device program
# --------------------------------------------------------------------------
def _build_program():
    import concourse.bass as bass
    import concourse.mybir as mybir
    import concourse.tile as tile
    from concourse import bacc

    dt = mybir.dt
    f32, bf16 = dt.float32, dt.bfloat16

    nc = bacc.Bacc("TRN2", target_bir_lowering=False, debug=False,
                   num_devices=NCORES)

    # ---- I/O ----
    xbf = nc.dram_tensor("xbf", [BT, C], bf16, kind="ExternalInput")
    x_own = nc.dram_tensor("x_own", [TSL, C], f32, kind="ExternalInput")
    wq2 = nc.dram_tensor("wq2", [C, 128], bf16, kind="ExternalInput")
    wk2 = nc.dram_tensor("wk2", [C, 128], bf16, kind="ExternalInput")
    wv2 = nc.dram_tensor("wv2", [C, 128], bf16, kind="ExternalInput")
    masks = nc.dram_tensor("masks", [128, 4, 512], bf16, kind="ExternalInput")
    wproj = nc.dram_tensor("wproj", [C, C], bf16, kind="ExternalInput")
    w1 = nc.dram_tensor("w1", [C, FF], bf16, kind="ExternalInput")
    w2 = nc.dram_tensor("w2", [FF, C], bf16, kind="ExternalInput")
    b1t = nc.dram_tensor("b1t", [128, FF // 128], f32, kind="ExternalInput")
    # rows blob: [ones(128) | bproj(1024) | b2row(1024)] bf16
    rows = nc.dram_tensor("rows", [1, 128 + C + C], bf16, kind="ExternalInput")
    out = nc.dram_tensor("out", [TSL, C], f32, kind="ExternalOutput")

    with tile.TileContext(nc, num_cores=NCORES) as tc:
        _body(nc, tc, tile, mybir, bass, locals())
    nc.compile()
    return nc


def _body(nc, tc, tile, mybir, bass, io):
    dt = mybir.dt
    f32, bf16 = dt.float32, dt.bfloat16
    AF = mybir.ActivationFunctionType
    OP = mybir.AluOpType

    xbf, x_own = io["xbf"], io["x_own"]
    wq2, wk2, wv2 = io["wq2"], io["wk2"], io["wv2"]
    masks, wproj = io["masks"], io["wproj"]
    w1, w2, b1t = io["w1"], io["w2"], io["b1t"]
    rows, out = io["rows"], io["out"]

    # ---- pools (released LIFO: persA first, dram last) ----
    dram = tc.alloc_tile_pool(name="dram", bufs=1, space="DRAM")
    consts = tc.alloc_tile_pool(name="consts", bufs=1)
    prefD = tc.alloc_tile_pool(name="prefD", bufs=1)   # phase-D persistents
    persA = tc.alloc_tile_pool(name="persA", bufs=1)   # attention lifetime

    # ---- phase-D prefetch ----
    wp_sb = prefD.tile([128, 8, C], bf16, name="wp_sb")
    xo = prefD.tile([128, 4, C], f32, name="xo")
    aT_own = prefD.tile([128, 8, 512], bf16, name="aT_own")
    rb = prefD.tile([128, 8, 512], bf16, name="rb")

    # ---- consts ----
    b1_sb = consts.tile([128, FF // 128], f32, name="b1_sb")
    rows_sb = consts.tile([1, 128 + C + C], bf16, name="rows_sb")
    mask_sb = consts.tile([128, 4, 512], bf16, name="mask_sb")
    wq_sb = consts.tile([128, 8, 128], bf16, name="wq_sb")
    wk_sb = consts.tile([128, 8, 128], bf16, name="wk_sb")
    wv_sb = consts.tile([128, 8, 128], bf16, name="wv_sb")
    eps_sb = consts.tile([128, 1], f32, name="eps_sb")
    ones_b = rows_sb[0:1, 0:128]
    bproj_r = rows_sb[0:1, 128:128 + C]
    b2_r = rows_sb[0:1, 128 + C:128 + 2 * C]

    def load_consts():
        nc.sync.dma_start(out=b1_sb[:], in_=b1t[:])
        nc.sync.dma_start(out=rows_sb[:], in_=rows[:])
        nc.sync.dma_start(out=mask_sb[:], in_=masks[:])
        nc.sync.dma_start(out=wq_sb[:],
                          in_=wq2[:].rearrange("(cc p) d -> p cc d", p=128))
        nc.sync.dma_start(out=wk_sb[:],
                          in_=wk2[:].rearrange("(cc p) d -> p cc d", p=128))
        nc.sync.dma_start(out=wv_sb[:],
                          in_=wv2[:].rearrange("(cc p) d -> p cc d", p=128))
        nc.vector.memset(eps_sb[:], EPS)

    # ---- attention persistents ----
    qT = [persA.tile([128, T], bf16, name=f"qTb{b}") for b in range(2)]
    kT = [persA.tile([128, T], bf16, name=f"kTb{b}") for b in range(2)]
    vaug = [persA.tile([128, 16, 130], bf16, name=f"vaugb{b}") for b in range(2)]
    aT_h = [[persA.tile([64, T], bf16, name=f"aTb{b}h{h}") for h in range(2)]
            for b in range(2)]
    dens = [persA.tile([65, 2, T], bf16, name=f"den_{b}") for b in range(2)]

    # DRAM scratch
    hdram = [[dram.tile([1024, C], bf16, name=f"hdram_{b}_{t}") for t in range(2)]
             for b in range(2)]
    vTdram = [dram.tile([128, T], bf16, name=f"vTdram_{b}") for b in range(2)]
    h2dram = [dram.tile([256, C], bf16, name=f"h2dram_{i}") for i in range(2)]
    denr = [dram.tile([1, 8, 512], bf16, name=f"denr_{h}") for h in range(2)]
    a2a_in = [dram.tile([8, 65, 512], bf16, name=f"a2a_in{h}") for h in range(2)]
    a2a_out = [dram.tile([8, 65, 512], bf16, name=f"a2a_out{h}")
               for h in range(2)]

    # ======================================================================
    # Phase A
    # ======================================================================
    with tc.tile_pool(name="lnA", bufs=1) as lnA, \
         tc.tile_pool(name="psA", bufs=1, space="PSUM") as psA:
        # ones columns of vaug (64 and 129)
        for b in range(2):
            nc.vector.memset(vaug[b][:, :, 64:65], 1.0)
            nc.vector.memset(vaug[b][:, :, 129:130], 1.0)

        # ---- LN1 -> h -> DRAM -> XBAR transpose -> QKV ----
        def qkv_chunk(b, tch):
            th, tl = tch // 2, tch % 2
            with nc.named_scope(f"qkv_b{b}t{tch}"):
                    row0 = b * T + tch * 512
                    xt = lnA.tile([128, 4, C], bf16, tag="xt", bufs=3,
                                  name=f"xt_{b}_{tch}")
                    nc.sync.dma_start(
                        out=xt[:],
                        in_=xbf[row0:row0 + 512, :].rearrange(
                            "(s p) c -> p s c", p=128))
                    if b == 0 and tch == 0:
                        load_consts()
                    ht = lnA.tile([128, 4, C], bf16, tag="ht", bufs=2,
                                  name=f"ht_{b}_{tch}")
                    for sub in range(4):
                        st = lnA.tile([128, 2, 6], f32, tag="st", bufs=2,
                                      name=f"st_{b}_{tch}_{sub}")
                        nc.vector.bn_stats(out=st[:, 0, :],
                                           in_=xt[:, sub, 0:512])
                        nc.vector.bn_stats(out=st[:, 1, :],
                                           in_=xt[:, sub, 512:1024])
                        mv = lnA.tile([128, 2], f32, tag="mv", bufs=2,
                                      name=f"mv_{b}_{tch}_{sub}")
                        nc.vector.bn_aggr(out=mv[:], in_=st[:])
                        rs = lnA.tile([128, 1], f32, tag="rs", bufs=2,
                                      name=f"rs_{b}_{tch}_{sub}")
                        nc.scalar.activation(out=rs[:], in_=mv[:, 1:2],
                                             func=AF.Sqrt, bias=eps_sb[:])
                        nc.vector.reciprocal(out=rs[:], in_=rs[:])
                        nmr = lnA.tile([128, 1], f32, tag="nmr", bufs=2,
                                       name=f"nmr_{b}_{tch}_{sub}")
                        nc.vector.tensor_scalar(
                            out=nmr[:], in0=mv[:, 0:1], scalar1=rs[:],
                            scalar2=-1.0, op0=OP.mult, op1=OP.mult)
                        nc.scalar.activation(
                            out=ht[:, sub, :], in_=xt[:, sub, :],
                            func=AF.Identity, bias=nmr[:], scale=rs[:])
                    nc.sync.dma_start(
                        out=hdram[b][th][tl * 512:tl * 512 + 512, :].rearrange(
                            "(s p) c -> p s c", p=128),
                        in_=ht[:])
                    # XBAR transpose: [512, 1024] -> [128, 8cc, 512] in one go
                    hTb = lnA.tile([128, 8, 512], bf16, tag="hTb", bufs=3,
                                   name=f"hTb_{b}_{tch}")
                    nc.sync.dma_start(
                        out=hTb[:],
                        in_=hdram[b][th][tl * 512:tl * 512 + 512, :],
                        transpose=True)
                    # q^T, k^T, v^T for this 512-token chunk
                    col = tch * 512
                    for w_sb, dstT in ((wq_sb, qT[b]), (wk_sb, kT[b])):
                        pqk = psA.tile([128, 512], f32, tag="pqk", bufs=2,
                                       name=f"pqk_{b}_{tch}_{dstT.name}")
                        for cc in range(8):
                            nc.tensor.matmul(pqk[:], w_sb[:, cc, :],
                                             hTb[:, cc, :],
                                             start=(cc == 0), stop=(cc == 7))
                        nc.vector.tensor_copy(out=dstT[:, col:col + 512],
                                              in_=pqk[:])
                    pvt = psA.tile([128, 512], f32, tag="pqk", bufs=2,
                                   name=f"pvt_{b}_{tch}")
                    for cc in range(8):
                        nc.tensor.matmul(pvt[:], wv_sb[:, cc, :],
                                         hTb[:, cc, :],
                                         start=(cc == 0), stop=(cc == 7))
                    vts = lnA.tile([128, 512], bf16, tag="vts", bufs=2,
                                   name=f"vts_{b}_{tch}")
                    nc.vector.tensor_copy(out=vts[:], in_=pvt[:])
                    nc.sync.dma_start(out=vTdram[b][:, col:col + 512],
                                      in_=vts[:])
                    # v back to token-major via XBAR, into vaug (65-strided)
                    vtmp = lnA.tile([128, 4, 128], bf16, tag="vtmp", bufs=2,
                                    name=f"vtmp_{b}_{tch}")
                    nc.sync.dma_start(out=vtmp[:],
                                      in_=vTdram[b][:, col:col + 512],
                                      transpose=True)
                    nc.vector.tensor_copy(
                        out=vaug[b][:, tch * 4:tch * 4 + 4, :].rearrange(
                            "p s (h x) -> p s h x", h=2)[:, :, :, 0:64],
                        in_=vtmp[:].rearrange("p s (h d) -> p s h d", h=2))
        # ---- attention, h-major; A2A#0 overlaps h=1 attention ----
        def receiver_chain(h):
            hp = 64 * h
            nc.sync.dma_start(
                out=aT_own[hp:hp + 64, :, :],
                in_=a2a_out[h][:, 0:64, :].rearrange("r d t -> d r t"))
            dsb = lnA.tile([8, 512], bf16, tag="dsb", bufs=2, name=f"dsb_{h}")
            nc.sync.dma_start(
                out=dsb[:],
                in_=a2a_out[h][:, 64:65, :].rearrange("r o t -> r (o t)"))
            with nc.allow_low_precision(reason="softmax denom recip in bf16"):
                nc.vector.reciprocal(out=dsb[:], in_=dsb[:])
            nc.sync.dma_start(out=denr[h][0, :, :], in_=dsb[:])
            nc.sync.dma_start(
                out=rb[hp:hp + 64, :, :],
                in_=denr[h][:].broadcast_to([64, 8, 512]))
            nc.vector.tensor_mul(aT_own[hp:hp + 64, :, :],
                                 aT_own[hp:hp + 64, :, :],
                                 rb[hp:hp + 64, :, :])

        def attn_half(h, b, half):
            hp = 64 * h
            den = dens[b]
            with nc.named_scope(f"attn_b{b}h{h}f{half}"):
                qc0 = half * 1024
                pat = [psA.tile([65, 512], f32, tag="pat", bufs=2,
                                name=f"pat_{b}_{h}_{half}_{i}")
                       for i in range(2)]
                nsb = 8 * half + 8
                for sb in range(nsb):
                    act0 = 0 if sb < 8 * half + 4 else 1
                    dtc = sb // 4 - 2 * half
                    ps = psA.tile([128, 1024], f32, tag="ps", bufs=2,
                                  name=f"ps_{b}_{h}_{half}_{sb}")
                    for i in range(act0, 2):
                        nc.tensor.matmul(
                            ps[:, i * 512:(i + 1) * 512],
                            kT[b][hp:hp + 64, sb * 128:sb * 128 + 128],
                            qT[b][hp:hp + 64,
                                  qc0 + i * 512:qc0 + (i + 1) * 512],
                            start=True, stop=True)
                    pt = lnA.tile([128, 1024], bf16, tag="pt", bufs=3,
                                  name=f"pt_{b}_{h}_{half}_{sb}")
                    nc.scalar.activation(out=pt[:, act0 * 512:1024],
                                         in_=ps[:, act0 * 512:1024],
                                         func=AF.Exp, scale=0.125)
                    if dtc >= act0:
                        nc.vector.tensor_mul(
                            pt[:, dtc * 512:(dtc + 1) * 512],
                            pt[:, dtc * 512:(dtc + 1) * 512],
                            mask_sb[:, sb % 4, :])
                    vs = h * 65
                    for i in range(act0, 2):
                        last = 8 * half + 3 if i == 0 else nsb - 1
                        nc.tensor.matmul(
                            pat[i][:], vaug[b][:, sb, vs:vs + 65],
                            pt[:, i * 512:(i + 1) * 512],
                            start=(sb == 0), stop=(sb == last))
                for i in range(2):
                    qcol = (2 * half + i) * 512
                    nc.vector.tensor_copy(
                        out=aT_h[b][h][:, qcol:qcol + 512],
                        in_=pat[i][0:64, :])
                    nc.vector.tensor_copy(
                        out=den[64:65, h, qcol:qcol + 512],
                        in_=pat[i][64:65, :])

        def stage(h, b):
            nc.sync.dma_start(
                out=a2a_in[h][4 * b:4 * b + 4, 0:64, :].rearrange(
                    "q d t -> d q t"),
                in_=aT_h[b][h][:].rearrange("d (q t) -> d q t", q=4))
            nc.sync.dma_start(
                out=a2a_in[h][4 * b:4 * b + 4, 64:65, :].rearrange(
                    "q o t -> o q t"),
                in_=dens[b][64:65, h, :].rearrange("o (q t) -> o q t", q=4))

        # QKV cross-batch interleaved with h=0 attention halves (two
        # independent LN chains keep the vector engine saturated; attention
        # fills PE bubbles)
        qkv_chunk(0, 0)
        qkv_chunk(1, 0)
        qkv_chunk(0, 1)
        qkv_chunk(1, 1)
        attn_half(0, 0, 0)
        qkv_chunk(0, 2)
        qkv_chunk(1, 2)
        # phase-D weight prefetch: transfers during attention
        nc.sync.dma_start(out=wp_sb[:],
                          in_=wproj[:].rearrange("(dc p) e -> p dc e", p=128))
        nc.sync.dma_start(out=xo[:],
                          in_=x_own[:].rearrange("(tq p) e -> p tq e", p=128))
        attn_half(0, 1, 0)
        qkv_chunk(0, 3)
        qkv_chunk(1, 3)
        attn_half(0, 0, 1)
        stage(0, 0)
        attn_half(0, 1, 1)
        stage(0, 1)
        nc.gpsimd.collective_compute(
            "AllToAll", mybir.AluOpType.bypass,
            replica_groups=[list(range(NCORES))],
            ins=[a2a_in[0][:].opt()], outs=[a2a_out[0][:].opt()])
        attn_half(1, 0, 0)
        attn_half(1, 0, 1)
        stage(1, 0)
        receiver_chain(0)  # A2A#0 landed during h1b0 attention
        attn_half(1, 1, 0)
        attn_half(1, 1, 1)
        stage(1, 1)
        nc.gpsimd.collective_compute(
            "AllToAll", mybir.AluOpType.bypass,
            replica_groups=[list(range(NCORES))],
            ins=[a2a_in[1][:].opt()], outs=[a2a_out[1][:].opt()])
        receiver_chain(1)

    # ---- projection (head-even half overlaps A2A#1) ----
    psP = tc.alloc_tile_pool(name="psP", bufs=1, space="PSUM")
    pp = [[psP.tile([128, 512], f32, tag="pp", bufs=8,
                    name=f"pp_{tq}_{eh}") for eh in range(2)]
          for tq in range(4)]
    for h in range(2):
        hp = 64 * h
        with nc.named_scope(f"proj_h{h}"):
            for tq in range(4):
                for eh in range(2):
                    for dc in range(8):
                        nc.tensor.matmul(
                            pp[tq][eh][:],
                            aT_own[hp:hp + 64, dc, tq * 128:(tq + 1) * 128],
                            wp_sb[hp:hp + 64, dc, eh * 512:eh * 512 + 512],
                            start=(h == 0 and dc == 0), stop=False)
                    if h == 1:
                        nc.tensor.matmul(
                            pp[tq][eh][:], ones_b,
                            bproj_r[0:1, eh * 512:eh * 512 + 512],
                            start=False, stop=True)
    persA.release()

    # ======================================================================
    # Phase D: residual + LN2 + FFN + residual
    # ======================================================================
    persD = tc.alloc_tile_pool(name="persD", bufs=1)
    x2 = persD.tile([128, 4, C], f32, name="x2")
    h2T = persD.tile([128, 8, 512], bf16, name="h2T")
    ff1T = persD.tile([128, 32, 512], bf16, name="ff1T")
    w1r = w1[:].rearrange("(cc p) m -> p cc m", p=128)
    with tc.tile_pool(name="prD", bufs=1) as prD:
        with nc.named_scope("ln2"):
            for tq in range(4):
                for eh in range(2):
                    nc.vector.tensor_add(x2[:, tq, eh * 512:(eh + 1) * 512],
                                         pp[tq][eh][:],
                                         xo[:, tq, eh * 512:(eh + 1) * 512])
                st2 = prD.tile([128, 2, 6], f32, tag="st2", bufs=2,
                               name=f"st2_{tq}")
                nc.vector.bn_stats(out=st2[:, 0, :], in_=x2[:, tq, 0:512])
                nc.vector.bn_stats(out=st2[:, 1, :], in_=x2[:, tq, 512:1024])
                mv2 = prD.tile([128, 2], f32, tag="mv2", bufs=2,
                               name=f"mv2_{tq}")
                nc.vector.bn_aggr(out=mv2[:], in_=st2[:])
                rs2 = prD.tile([128, 1], f32, tag="rs2", bufs=2,
                               name=f"rs2_{tq}")
                nc.scalar.activation(out=rs2[:], in_=mv2[:, 1:2], func=AF.Sqrt,
                                     bias=eps_sb[:])
                nc.vector.reciprocal(out=rs2[:], in_=rs2[:])
                h2 = prD.tile([128, C], bf16, tag="h2", bufs=2, name=f"h2_{tq}")
                nc.vector.tensor_scalar(out=h2[:], in0=x2[:, tq, :],
                                        scalar1=mv2[:, 0:1], scalar2=rs2[:],
                                        op0=OP.subtract, op1=OP.mult)
                nc.sync.dma_start(
                    out=h2dram[tq // 2][(tq % 2) * 128:(tq % 2) * 128 + 128, :],
                    in_=h2[:])
                if tq % 2 == 1:
                    h2Ttmp = prD.tile([128, 8, 256], bf16, tag="h2Ttmp",
                                      bufs=2, name=f"h2Ttmp_{tq}")
                    nc.scalar.dma_start(out=h2Ttmp[:],
                                        in_=h2dram[tq // 2][:],
                                        transpose=True)
                    nc.vector.tensor_copy(
                        out=h2T[:, :, (tq // 2) * 256:(tq // 2) * 256 + 256],
                        in_=h2Ttmp[:])
        psP.release()
        with tc.tile_pool(name="ps1", bufs=1, space="PSUM") as ps1, \
             nc.named_scope("ffn1"):
            for w in range(16):  # m-windows of 256
                w1w = prD.tile([128, 8, 256], bf16, tag="w1w", bufs=3,
                               name=f"w1w_{w}")
                nc.sync.dma_start(out=w1w[:],
                                  in_=w1r[:, :, w * 256:(w + 1) * 256])
                for m2 in range(2):
                    m = w * 2 + m2
                    pf = ps1.tile([128, 512], f32, tag="pf", bufs=3,
                                  name=f"pf_{m}")
                    for cc in range(8):
                        nc.tensor.matmul(
                            pf[:], w1w[:, cc, m2 * 128:(m2 + 1) * 128],
                            h2T[:, cc, :], start=(cc == 0), stop=(cc == 7))
                    nc.scalar.activation(out=ff1T[:, m, :], in_=pf[:],
                                         func=AF.Relu, bias=b1_sb[:, m:m + 1])
        with tc.tile_pool(name="ps2", bufs=1, space="PSUM") as ps2p, \
             nc.named_scope("ffn2"):
            pso = [ps2p.tile([128, C], f32, tag="pso", bufs=4, name=f"pso_{tq}")
                   for tq in range(4)]
            for mc in range(32):
                w2t = prD.tile([128, C], bf16, tag="w2t", bufs=8,
                               name=f"w2t_{mc}")
                nc.sync.dma_start(out=w2t[:],
                                  in_=w2[mc * 128:(mc + 1) * 128, :])
                for tq in range(4):
                    for eh in range(2):
                        nc.tensor.matmul(pso[tq][:, eh * 512:(eh + 1) * 512],
                                         ff1T[:, mc, tq * 128:(tq + 1) * 128],
                                         w2t[:, eh * 512:(eh + 1) * 512],
                                         start=(mc == 0), stop=False)
            for tq in range(4):
                for eh in range(2):
                    nc.tensor.matmul(pso[tq][:, eh * 512:(eh + 1) * 512],
                                     ones_b, b2_r[0:1, eh * 512:(eh + 1) * 512],
                                     start=False, stop=True)
                ot = prD.tile([128, C], f32, tag="ot", bufs=2, name=f"ot_{tq}")
                nc.vector.tensor_add(ot[:], pso[tq][:], x2[:, tq, :])
                nc.sync.dma_start(out=out[tq * 128:(tq + 1) * 128, :],
                                  in_=ot[:])
    persD.release()
    prefD.release()
    consts.release()
    dram.release()


# --------------------------------------------------------------------------
# host driver
# --------------------------------------------------------------------------
def _make_in_maps(inputs):
    bf = ml_dtypes.bfloat16
    x = np.ascontiguousarray(np.asarray(inputs["x"], np.float32))
    wq = np.asarray(inputs["wq"], np.float32)
    wk = np.asarray(inputs["wk"], np.float32)
    wv = np.asarray(inputs["wv"], np.float32)
    w_proj = np.asarray(inputs["w_proj"], np.float32)
    b_proj = np.asarray(inputs["b_proj"], np.float32)
    w1 = np.asarray(inputs["w1"], np.float32)
    b1 = np.asarray(inputs["b1"], np.float32)
    w2 = np.asarray(inputs["w2"], np.float32)
    b2 = np.asarray(inputs["b2"], np.float32)
    g1 = np.asarray(inputs["g1"], np.float32)
    be1 = np.asarray(inputs["be1"], np.float32)
    g2 = np.asarray(inputs["g2"], np.float32)
    be2 = np.asarray(inputs["be2"], np.float32)

    assert np.max(np.abs(be1)) == 0.0, "be1 != 0 unsupported in this build"
    xf = x.reshape(BT, C)
    i_mask = np.zeros((128, 4, 512), np.float32)
    s_idx = np.arange(128)[:, None]
    t_idx = np.arange(512)[None, :]
    for i in range(4):
        i_mask[:, i, :] = (s_idx + 128 * i <= t_idx).astype(np.float32)

    # fold LN affine into weights
    wq_g = g1[None, :, None] * wq  # (H, C, D) scaled along C
    wk_g = g1[None, :, None] * wk
    wv_g = g1[None, :, None] * wv
    w1_g = g2[:, None] * w1
    b1_eff = b1 + be2 @ w1

    rows_blob = np.concatenate(
        [np.ones(128, np.float32), b_proj, b2]).astype(bf)[None, :]

    common = dict(
        xbf=xf.astype(bf),
        masks=i_mask.astype(bf),
        wproj=w_proj.astype(bf),
        w1=w1_g.astype(bf), w2=w2.astype(bf),
        b1t=np.ascontiguousarray(b1_eff.reshape(FF // 128, 128).T),
        rows=np.ascontiguousarray(rows_blob),
    )
    in_maps = []
    for c in range(NCORES):
        b, hg = c // 4, c % 4
        m = dict(common)
        m["x_own"] = np.ascontiguousarray(
            xf[b * T + hg * TSL: b * T + (hg + 1) * TSL])
        m["wq2"] = np.ascontiguousarray(
            np.concatenate([wq_g[2 * c], wq_g[2 * c + 1]], axis=1)).astype(bf)
        m["wk2"] = np.ascontiguousarray(
            np.concatenate([wk_g[2 * c], wk_g[2 * c + 1]], axis=1)).astype(bf)
        m["wv2"] = np.ascontiguousarray(
            np.concatenate([wv_g[2 * c], wv_g[2 * c + 1]], axis=1)).astype(bf)
        in_maps.append(m)
    return in_maps


LAST_RESULTS = None


def kernel(trace=False, **inputs):
    global LAST_RESULTS
    from concourse import bass_utils

    if "nc" not in _CACHE:
        _CACHE["nc"] = _build_program()
    nc = _CACHE["nc"]
    in_maps = _make_in_maps(inputs)
    res = bass_utils.run_bass_kernel_spmd(
        nc, in_maps, core_ids=list(range(NCORES)), trace=trace)
    LAST_RESULTS = res
    out = np.zeros((B, T, C), np.float32)
    for c in range(NCORES):
        b, hg = c // 4, c % 4
        out[b, hg * TSL:(hg + 1) * TSL, :] = res.results[c]["out"]
    return out
